# revision 35
# baseline (speedup 1.0000x reference)
"""Trainium2 Bass kernel for nn_ConditionedDense (hypernetwork-conditioned dense).

Reference computation:
    A = einsum('bnp,pq->bnq', P, Wk)         # hypernetwork: per-position weights
    W = relu(A).reshape(B, N, c_in, c_out)
    out = einsum('bni,bnio->bno', X, W)

Strategy (v21): pure data parallel over 8 NeuronCores (shard batch dim),
A^T-oriented dataflow so both einsums run on the PE with static weights:

  - A^T layout: [q' partitions, pos free] with q' = o*32 + i.  PE computes
    A^T chunk pairs (2 x 128 q' x 512 pos) with lhsT = Wk' chunks (static),
    rhs = P^T.  K=64 -> the pair runs row-tiled (rows 0-63 / 64-127), with
    P^T duplicated on partitions 64-127.
  - m = relu(A) * X, per chunk-pair-unit path (22 D / 10 S per 32 units,
    phase-aligned, chosen empirically; D-unit multiplies are emitted with
    a one-unit lag so the in-order DVE never stalls on a just-issued relu):
      D unit: scalar.activation(Relu) PSUM->SBUF bf16, then DVE
              tensor_tensor mult (2x bf16) by X replicated 4x on
              partitions (X_rep[p,t] = X[t, p%32], chunk-independent).
              Adjacent-j D units in one phase share a [128,2,2T] w/m pair
              so their two mults merge into one DVE op (amortizes the
              ~100ns DVE op overhead and halves sem traffic).
      S unit: fused DVE scalar_tensor_tensor (max 0, mult) from PSUM (1x).
      (A GPSIMD "G" path exists but measured slower: gpsimd tensor_tensor
      is ~2.2us/unit and contends with DVE for the shared SBUF port,
      degrading the 2x TT mults ~50%; left for experiments via K_PATHS.)
    This mix saturates both ACT (~105us busy: 88 relus at (1024+352)/1.2ns
    + 8 out drains) and DVE (~112us: 44 merged TTs + 40 STTs + sems) -- the
    measured floor for any PSUM-materializing dataflow here, since only
    ACT/DVE can read PSUM and both are ~1 elem/lane/cycle on this stream.
  - reduce over i on the PE: 8 accumulating matmuls per tile with static
    0/1 selection weights S_c[p, o] = (o == 4c + p//32), output col-tiled
    into out^T PSUM [32j:32j+32, :] per supergroup of 4 tiles; bursts are
    emitted lagged one chunk-pair phase so the in-order PE always has the
    next phase's A-gen matmuls queued ahead of a burst that would
    otherwise idle waiting for the slowest unit's m.
  - out^T drained by ACT copies (deferred one SG to avoid stalling the
    relu stream), DMA'd in packed transposed layout; host unpacks (free).
  - DMA: one trigger per tensor per SG (descriptors fan out across all 16
    queues; Sync DIRECT2D triggers cost ~600ns each so fewer is better).
    SG0's first-needed slices trigger from ACT/GPSIMD/Sync in parallel for
    a fast pipeline fill; later SGs prefetch at the previous SG's phase 2;
    each SG's out^T drains right after its last (lagged) reduce burst to
    keep the endgame tail short.

Host side (free): P^T duplicated x2, X^T replicated x4, Wk column-permuted
to q' = o*32+i and packed into row-tiled pairs, S selection matrices, all
cast to bf16.
"""

import os
from contextlib import ExitStack

import numpy as np
import ml_dtypes

import concourse.bass as bass
import concourse.tile as tile
from concourse import bacc, mybir
from concourse.bass_utils import run_bass_kernel_spmd

C_IN = 32
C_OUT = 32
P_DIM = 64
Q = C_IN * C_OUT             # 1024
B, N = 32, 4096
N_CORES = 8
B_SH = B // N_CORES          # 4 batches per core
NPOS = B_SH * N              # 16384 positions per core
T = 512                      # positions per tile (matmul N)
TILES = NPOS // T            # 32
SG_TILES = 4                 # tiles per supergroup (col-tiled out^T group)
N_SG = TILES // SG_TILES     # 8
T_SG = T * SG_TILES          # 2048 positions per supergroup
PAIRS = 4                    # chunk pairs per tile (8 q'-chunks of 128)
# per chunk-pair-unit m-production path, indexed by unit_idx % 32:
#   D = ACT relu -> DVE tensor_tensor mult (2x bf16)
#   S = fused DVE scalar_tensor_tensor (relu+mult) straight from PSUM
UNIT_PATHS = os.environ.get(
    "K_PATHS", "DDSDDDSSDDSDDDSDDDSDDDSSDDSDDDSD"
)

F32 = mybir.dt.float32
BF16 = mybir.dt.bfloat16

_BUILD_CACHE = {}
LAST_RESULTS = None  # BassKernelResults of the most recent run (for profiling)


def _build_nc():
    nc = bacc.Bacc(
        "TRN2", target_bir_lowering=False, debug=False, num_devices=N_CORES
    )
    XR_d = nc.declare_dram_parameter("XR", [N_SG * 128, T_SG], BF16, isOutput=False)
    P2_d = nc.declare_dram_parameter("P2", [N_SG * 128, T_SG], BF16, isOutput=False)
    WK_d = nc.declare_dram_parameter("WK", [128, PAIRS * 128], BF16, isOutput=False)
    S_d = nc.declare_dram_parameter("S", [128, 8 * C_OUT], BF16, isOutput=False)
    out_d = nc.declare_dram_parameter("out", [N_SG * 128, T], BF16, isOutput=True)

    relu = mybir.ActivationFunctionType.Relu
    copyf = mybir.ActivationFunctionType.Copy
    mult = mybir.AluOpType.mult
    amax = mybir.AluOpType.max

    with ExitStack() as ctx:
        tc = ctx.enter_context(tile.TileContext(nc))
        wkp = ctx.enter_context(tc.tile_pool(name="wk", bufs=1))
        ssp = ctx.enter_context(tc.tile_pool(name="sel", bufs=1))
        xrp = ctx.enter_context(
            tc.tile_pool(name="xr", bufs=int(os.environ.get("K_INBUFS", "3")))
        )
        p2p = ctx.enter_context(
            tc.tile_pool(name="p2", bufs=int(os.environ.get("K_INBUFS", "3")))
        )
        apool = ctx.enter_context(tc.tile_pool(name="apsum", bufs=3, space="PSUM"))
        megap = ctx.enter_context(tc.tile_pool(name="amega", bufs=1, space="PSUM"))
        wpool = ctx.enter_context(
            tc.tile_pool(name="w", bufs=int(os.environ.get("K_WBUFS", "4")))
        )
        mpool = ctx.enter_context(
            tc.tile_pool(name="m", bufs=int(os.environ.get("K_MBUFS", "6")))
        )
        w2pool = ctx.enter_context(
            tc.tile_pool(name="w2", bufs=int(os.environ.get("K_W2BUFS", "4")))
        )
        m2pool = ctx.enter_context(
            tc.tile_pool(name="m2", bufs=int(os.environ.get("K_M2BUFS", "7")))
        )
        w3pool = ctx.enter_context(
            tc.tile_pool(name="w3", bufs=int(os.environ.get("K_W3BUFS", "4")))
        )
        m3pool = ctx.enter_context(
            tc.tile_pool(name="m3", bufs=int(os.environ.get("K_M3BUFS", "6")))
        )
        opool = ctx.enter_context(tc.tile_pool(name="opsum", bufs=2, space="PSUM"))
        obp = ctx.enter_context(tc.tile_pool(name="osb", bufs=2))

        wk_t = wkp.tile([128, PAIRS, 128], BF16)
        s_t = ssp.tile([128, 8, C_OUT], BF16)
        MEGA = os.environ.get("K_MEGA", "0") == "1"
        if MEGA:
            mega = megap.tile([128, 6, T], F32)
        else:
            mega = None
        slot_cnt = [0]

        self_cnt = [0]
        LAG = int(os.environ.get("K_LAG", "1"))
        pend_mult = []       # FIFO of deferred DVE tensor_tensor mults
        pend_gp = [None]     # deferred GPSIMD tensor_tensor mult (G path)
        pend_reduce = [None]
        pending_store = []

        def flush_pend(limit=0):
            while len(pend_mult) > limit:
                pm, pw, px = pend_mult.pop(0)
                nc.vector.tensor_tensor(out=pm[:], in0=pw[:], in1=px, op=mult)
            if pend_gp[0] is not None:
                pm, pw, px = pend_gp[0]
                nc.gpsimd.tensor_tensor(out=pm[:], in0=pw[:], in1=px, op=mult)
                pend_gp[0] = None
        sg_tiles = {}

        def fetch_sg(s):
            """Allocate + fetch the input tiles for supergroup s (prefetch)."""
            if s >= N_SG or s in sg_tiles:
                return
            xr = xrp.tile([128, T_SG], BF16)
            p2 = p2p.tile([128, T_SG], BF16)
            r0 = s * 128
            nc.sync.dma_start(out=p2[:], in_=P2_d[r0:r0 + 128, :])
            nc.sync.dma_start(out=xr[:], in_=XR_d[r0:r0 + 128, :])
            sg_tiles[s] = (xr, p2)

        for sg in range(N_SG):
            if sg == 0:
                xr = xrp.tile([128, T_SG], BF16)
                p2 = p2p.tile([128, T_SG], BF16)
                sg_tiles[0] = (xr, p2)
                r0 = 0
                # fast start: the first matmul needs wk pair 0 + p2's first
                # tile slice.  Each DMA trigger costs ~600ns of *serial*
                # engine time, so fan the initial triggers out across the
                # (idle) compute engines to cut the pipeline fill.
                h0 = bass.ts(0, T)
                nc.scalar.dma_start(
                    out=wk_t[:, 0, :], in_=WK_d[:, 0:128]
                )
                if os.environ.get("K_GPDMA", "0") == "1":
                    nc.gpsimd.dma_start(
                        out=p2[0:64, h0], in_=P2_d[r0:r0 + 64, h0]
                    )
                else:
                    nc.sync.dma_start(
                        out=p2[0:64, h0], in_=P2_d[r0:r0 + 64, h0]
                    )
                nc.sync.dma_start(
                    out=p2[64:128, h0], in_=P2_d[r0 + 64:r0 + 128, h0]
                )
                nc.sync.dma_start(out=xr[:, h0], in_=XR_d[r0:r0 + 128, h0])
                nc.sync.dma_start(
                    out=s_t[:], in_=S_d[:].rearrange("p (a b) -> p a b", a=8)
                )
                nc.sync.dma_start(
                    out=wk_t[:, 1:PAIRS, :], in_=WK_d[:, 128:].rearrange(
                        "p (a b) -> p a b", a=PAIRS - 1
                    )
                )
                rest = slice(T, T_SG)
                nc.sync.dma_start(out=p2[:, rest], in_=P2_d[r0:r0 + 128, rest])
                nc.sync.dma_start(out=xr[:, rest], in_=XR_d[r0:r0 + 128, rest])
            fetch_sg(sg)
            xr, p2 = sg_tiles.pop(sg)

            ot = opool.tile([128, T], F32)
            # SG-wide chunk-pair phases: all 4 tiles' units per pair, then
            # an 8-matmul reduce burst whose adjacent col-tiled matmuls
            # (4 col groups) run concurrently on the PE
            for p in range(PAIRS):
                paths = [
                    UNIT_PATHS[(self_cnt[0] + j) % len(UNIT_PATHS)]
                    for j in range(SG_TILES)
                ]
                if sg == 0 and p == 0 and os.environ.get("K_S0", "0") == "1":
                    # pipeline fill: an S unit first lets the DVE start on
                    # the very first psA instead of waiting out ACT's first
                    # relu; mid-stream phases keep the balanced pattern
                    paths = (
                        ["S", "S", "D", "D"]
                        if os.environ.get("K_S0P", "SSDD") == "SSDD"
                        else ["S", "D", "D", "D"]
                    )
                if (
                    sg == N_SG - 1 and p == PAIRS - 1
                    and os.environ.get("K_SLAST", "0") == "1"
                ):
                    # pipeline drain: end on a pair + two S units so the
                    # final burst's slowest m arrives as early as possible
                    # (no lagged solo-D mult at the very end)
                    paths = ["D", "D", "S", "S"]
                self_cnt[0] += SG_TILES
                # merge runs of adjacent-j D units: a run's DVE multiplies
                # merge into one [128, 2, len*T] tensor_tensor (better rate,
                # fewer sems; the shared w/m tiles span the run's positions)
                run_of = {}
                jj = 0
                while jj < SG_TILES:
                    if paths[jj] == "D":
                        ln = 1
                        while jj + ln < SG_TILES and paths[jj + ln] == "D":
                            ln += 1
                        for k in range(ln):
                            run_of[jj + k] = (jj, ln)
                        jj += ln
                    else:
                        jj += 1
                rpools = {1: (wpool, mpool), 2: (w2pool, m2pool),
                          3: (w3pool, m3pool)}
                units = []
                shared = {}
                jorder = list(range(SG_TILES))
                if os.environ.get("K_JORDER", "") == "s2nd":
                    # emit the S unit's A-gens second so the DVE's fused op
                    # gets its PSUM input one matmul earlier each phase,
                    # while ACT still starts immediately on j0's relu
                    s_js = [j for j in jorder if paths[j] == "S"]
                    if len(s_js) == 1 and s_js[0] != 0:
                        jorder.remove(s_js[0])
                        jorder.insert(1, s_js[0])
                for j in jorder:
                    js = bass.ts(j, T)
                    if MEGA:
                        s_ = slot_cnt[0] % 3
                        slot_cnt[0] += 1
                        psA = mega[:, 2 * s_:2 * s_ + 2, :]
                    else:
                        psA = apool.tile([128, 2, T], F32)
                    nc.tensor.matmul(
                        psA[:, 0, :], lhsT=wk_t[0:64, p, :],
                        rhs=p2[0:64, js], start=True, stop=True,
                    )
                    nc.tensor.matmul(
                        psA[:, 1, :], lhsT=wk_t[64:128, p, :],
                        rhs=p2[64:128, js], start=True, stop=True,
                    )
                    x_in = xr[:, js].unsqueeze(1).broadcast_to(
                        [128, 2, T]
                    )
                    path = paths[j]
                    if path == "S":
                        m = mpool.tile([128, 2, T], BF16)
                        nc.vector.scalar_tensor_tensor(
                            out=m[:], in0=psA[:], scalar=0.0,
                            in1=x_in, op0=amax, op1=mult,
                        )
                        flush_pend(limit=max(0, LAG - 1))
                        units.append((j, m, 0))
                    elif path == "D" and run_of[j][1] > 1:
                        st, ln = run_of[j]
                        if j == st:
                            wp_, mp_ = rpools[min(ln, 3)]
                            if ln > 3:
                                wp_, mp_ = rpools[3]
                            wr = wp_.tile([128, 2, ln * T], BF16)
                            mr = mp_.tile([128, 2, ln * T], BF16)
                            shared[st] = (wr, mr)
                        wr, mr = shared[st]
                        lo = j - st
                        nc.scalar.activation(
                            wr[:, :, lo * T:(lo + 1) * T], psA[:], relu
                        )
                        if j == st + ln - 1:
                            xrun = xr[:, st * T:(st + ln) * T].unsqueeze(
                                1
                            ).broadcast_to([128, 2, ln * T])
                            pend_mult.append((mr, wr, xrun))
                            flush_pend(limit=LAG)
                        else:
                            flush_pend(limit=max(0, LAG - 1))
                        units.append((j, mr, lo))
                    else:
                        m = mpool.tile([128, 2, T], BF16)
                        w = wpool.tile([128, 2, T], BF16)
                        nc.scalar.activation(w[:], psA[:], relu)
                        # lag this unit's mult so the relu above has time
                        # to finish before the consumer pops it
                        if path == "G":
                            flush_pend(limit=max(0, LAG - 1))
                            pend_gp[0] = (m, w, x_in)
                        else:
                            pend_mult.append((m, w, x_in))
                            flush_pend(limit=LAG)
                        units.append((j, m, 0))
                flush_pend(limit=0 if os.environ.get("K_PHFLUSH", "1") == "1" else LAG - 1)
                # lag the reduce burst one phase: the PE then has the next
                # phase's A-gen matmuls queued ahead of a burst that would
                # otherwise idle waiting for the last unit's m
                if pend_reduce[0] is not None:
                    pp, punits, pot = pend_reduce[0]
                    for c2 in range(2):
                        chunk = 2 * pp + c2
                        for (j, m, lo) in punits:
                            nc.tensor.matmul(
                                pot[32 * j:32 * (j + 1), :],
                                lhsT=s_t[:, chunk, :],
                                rhs=m[:, c2, lo * T:(lo + 1) * T],
                                start=(chunk == 0), stop=(chunk == 7),
                                tile_position=(0, 32 * j),
                            )
                pend_reduce[0] = (p, units, ot)
                if p == int(os.environ.get("K_DRAINP", "0")) and pending_store:
                    # the lagged burst just emitted was the previous SG's
                    # last one -> its out^T is complete; drain it now so the
                    # final-SG tail stays short
                    psg, pot = pending_store.pop(0)
                    osb = obp.tile([128, T], BF16)
                    nc.scalar.activation(osb[:], pot[:], copyf)
                    nc.sync.dma_start(
                        out=out_d[psg * 128:(psg + 1) * 128, :], in_=osb[:]
                    )
                if p == 2:
                    fetch_sg(sg + 1)
                if p == 3 and os.environ.get("K_PF2", "0") == "1":
                    fetch_sg(sg + 2)

            pending_store.append((sg, ot))
        if pend_reduce[0] is not None:
            pp, punits, pot = pend_reduce[0]
            for c2 in range(2):
                chunk = 2 * pp + c2
                for (j, m, lo) in punits:
                    nc.tensor.matmul(
                        pot[32 * j:32 * (j + 1), :],
                        lhsT=s_t[:, chunk, :],
                        rhs=m[:, c2, lo * T:(lo + 1) * T],
                        start=(chunk == 0), stop=(chunk == 7),
                        tile_position=(0, 32 * j),
                    )
            pend_reduce[0] = None
        for psg, pot in pending_store:
            osb = obp.tile([128, T], BF16)
            nc.scalar.activation(osb[:], pot[:], copyf)
            nc.sync.dma_start(
                out=out_d[psg * 128:(psg + 1) * 128, :], in_=osb[:]
            )

    nc.finalize()
    return nc


def _get_nc():
    key = "v20-" + UNIT_PATHS
    if key not in _BUILD_CACHE:
        _BUILD_CACHE[key] = _build_nc()
    return _BUILD_CACHE[key]


def _host_prep(X, P, Wk):
    """Build per-core input arrays (host-side prep is free)."""
    bf16 = ml_dtypes.bfloat16
    # Wk' with q' = o*32 + i
    WkP = np.ascontiguousarray(
        Wk.reshape(P_DIM, C_IN, C_OUT).transpose(0, 2, 1).reshape(P_DIM, Q)
    )
    # packed row-tiled pairs: [128, PAIRS, 128] -> [128, PAIRS*128]
    wk2 = np.zeros((128, PAIRS, 128), dtype=np.float32)
    for p in range(PAIRS):
        wk2[0:64, p, :] = WkP[:, 256 * p:256 * p + 128]
        wk2[64:128, p, :] = WkP[:, 256 * p + 128:256 * p + 256]
    WK_h = np.ascontiguousarray(wk2.reshape(128, PAIRS * 128)).astype(bf16)

    # S selection: S[pr, c, o] = 1 if o == 4c + pr//32
    pr = np.arange(128)[:, None, None]
    cc = np.arange(8)[None, :, None]
    oo = np.arange(C_OUT)[None, None, :]
    S = (oo == 4 * cc + pr // 32).astype(np.float32)
    S_h = np.ascontiguousarray(S.reshape(128, 8 * C_OUT)).astype(bf16)

    in_maps = []
    for c in range(N_CORES):
        Xc = np.ascontiguousarray(
            X[c * B_SH:(c + 1) * B_SH].reshape(NPOS, C_IN)
        )
        Pc = np.ascontiguousarray(
            P[c * B_SH:(c + 1) * B_SH].reshape(NPOS, P_DIM)
        )
        # X_rep [128, NPOS]: row pr = X[:, pr % 32]; then [sg] blocks
        XRc = np.tile(Xc.T, (4, 1))                    # [128, NPOS]
        XR_h = np.ascontiguousarray(
            XRc.reshape(128, N_SG, T_SG).transpose(1, 0, 2).reshape(
                N_SG * 128, T_SG
            )
        ).astype(bf16)
        # P^T duplicated x2 on partitions
        P2c = np.tile(Pc.T, (2, 1))                    # [128, NPOS]
        P2_h = np.ascontiguousarray(
            P2c.reshape(128, N_SG, T_SG).transpose(1, 0, 2).reshape(
                N_SG * 128, T_SG
            )
        ).astype(bf16)
        in_maps.append({"XR": XR_h, "P2": P2_h, "WK": WK_h, "S": S_h})
    return in_maps


def kernel(X, P, Wk):
    global LAST_RESULTS
    X = np.asarray(X, dtype=np.float32)
    P = np.asarray(P, dtype=np.float32)
    Wk = np.asarray(Wk, dtype=np.float32)

    in_maps = _host_prep(X, P, Wk)

    nc = _get_nc()
    trace = os.environ.get("BASS_PROFILE", "0") == "1"
    kw = {}
    if os.environ.get("BASS_TMPDIR"):
        kw["tmpdir"] = os.environ["BASS_TMPDIR"]
    res = run_bass_kernel_spmd(
        nc, in_maps, list(range(N_CORES)), trace=trace, **kw
    )
    LAST_RESULTS = res

    out = np.empty((B, N, C_OUT), dtype=np.float32)
    for c in range(N_CORES):
        # packed out^T (f32): [sg*128 + 32j + o, t] -> pos = sg*T_SG + j*T + t
        o_c = (
            np.asarray(res.results[c]["out"])
            .astype(np.float32)
            .reshape(N_SG, SG_TILES, C_OUT, T)
            .transpose(0, 1, 3, 2)
            .reshape(B_SH, N, C_OUT)
        )
        out[c * B_SH:(c + 1) * B_SH] = o_c
    return out

